# revision 5
# baseline (speedup 1.0000x reference)
"""BertSelfAttention forward on 8 Trainium2 NeuronCores (Bass/Tile).

Problem: B=4, S=2048, D=1024, H=16 heads, head_dim=64.
Sharding: 8 cores = (batch b in 0..4) x (head-group g in 0..2); each core
computes heads [8g, 8g+8) of batch b => output slice out[b, :, 512g:512(g+1)].

Per-core kernel (all matmuls in float32r: full PE rate, ~1e-4 rel err):
  phase 1: qT = (wq @ x.T) + bq      [dh=512, S]   (dh on partitions)
           kT = (wk @ x.T) + bk      [dh=512, S]
           v  = (x @ wv.T + bv)*pad  [S, dh=512]   stored per s-tile as
                [128, 8 heads, 65] with col 64 = pad (ones) column -> PV
                matmul emits softmax denominators for free.
  phase 2: per head-pair (2p, 2p+1) sharing the PE array via row-tiling
           (each head has contraction K=64 -> partitions 0:64 / 64:128):
           scoresT[ks, qs] = kT_h.T @ qT_h  (PSUM, 2 k-tiles per group)
           e = exp(0.125 * scoresT)         (one ScalarE pass, PSUM->SBUF)
           ctxT_aug[65, qs] += v_aug.T @ e  (accumulate over 16 k-tiles)
           PE-transpose 128-blocks of ctxT_aug, divide rows by the
           denominator column (VectorE reciprocal + tensor_scalar), DMA out.

The mask is exact: exp(s + (1-pad)*-1e9) = exp(s)*pad, folded into v rows
and the denominator column (both multiplied by pad).
"""

import numpy as np

import concourse.bass as bass
import concourse.tile as tile
from concourse import mybir
from concourse.bass import ds, ts
from concourse.masks import make_identity

B, S, D, H = 4, 2048, 1024, 16
HD = D // H          # 64
DH = 512             # head dims per core (8 heads)
NHEADS = 8           # heads per core
NPAIRS = 4           # head pairs per core
KT = 16              # k-tiles of 128 over S
QC = 4               # q-chunks of 512 over S
KD = 8               # contraction tiles of 128 over D
GS = 2               # k-tiles per exp group (2 PSUM banks)

F32 = mybir.dt.float32
F32R = mybir.dt.float32r

MAX_WAITS = 1


def split_excess_waits(nc):
    """This walrus build accepts only one sync-wait per instruction; hoist
    excess waits onto preceding NoOps on the same engine."""
    for f in nc.m.functions:
        for bb in f.blocks:
            insts = list(bb.instructions)
            out = []
            changed = False
            for inst in insts:
                si = inst.sync_info
                if si is not None and si.on_wait and len(si.on_wait) > MAX_WAITS:
                    waits = list(si.on_wait)
                    changed = True
                    k = 0
                    while len(waits) - k > MAX_WAITS:
                        nop = mybir.InstNoOp(
                            name=f"{inst.name}-ws{k}", engine=inst.engine
                        )
                        nop.sync_info = mybir.SyncInfo(
                            on_wait=waits[k : k + MAX_WAITS], on_update=[]
                        )
                        out.append(nop)
                        k += MAX_WAITS
                    si.on_wait = waits[k:]
                out.append(inst)
            if changed:
                bb.instructions = out


def build_nc():
    nc = bass.Bass("TRN2", target_bir_lowering=False, debug=False, num_devices=8)

    xT_d = nc.declare_dram_parameter("xT", [D, S], F32R, isOutput=False)
    wqT_d = nc.declare_dram_parameter("wqT", [D, DH], F32R, isOutput=False)
    wkT_d = nc.declare_dram_parameter("wkT", [D, DH], F32R, isOutput=False)
    wvT_d = nc.declare_dram_parameter("wvT", [D, DH], F32R, isOutput=False)
    bq_d = nc.declare_dram_parameter("bq", [DH], F32, isOutput=False)
    bk_d = nc.declare_dram_parameter("bk", [DH], F32, isOutput=False)
    bv_d = nc.declare_dram_parameter("bv", [DH], F32, isOutput=False)
    pad_d = nc.declare_dram_parameter("pad", [S], F32, isOutput=False)
    out_d = nc.declare_dram_parameter("ctx", [S, DH], F32, isOutput=True)

    with tile.TileContext(nc) as tc:
        with (
            tc.tile_pool(name="persist", bufs=1) as persist,
            tc.tile_pool(name="small", bufs=1) as small,
        ):
            qT_sb = persist.tile([128, 4, S], F32R)
            kT_sb = persist.tile([128, 4, S], F32R)
            v_sb = persist.tile([128, KT, NHEADS, HD + 1], F32R)
            bq_sb = small.tile([128, 4], F32)
            bk_sb = small.tile([128, 4], F32)
            pad_sb = small.tile([128, KT], F32)
            bv_bc = small.tile([128, DH], F32)
            ident = small.tile([128, 128], F32)

            nc.gpsimd.dma_start(out=bq_sb, in_=bq_d.ap().rearrange("(m p) -> p m", p=128))
            nc.gpsimd.dma_start(out=bk_sb, in_=bk_d.ap().rearrange("(m p) -> p m", p=128))
            nc.gpsimd.dma_start(out=pad_sb, in_=pad_d.ap().rearrange("(t p) -> p t", p=128))
            nc.gpsimd.dma_start(
                out=bv_bc,
                in_=bass.AP(tensor=bv_d, offset=0, ap=[[0, 128], [1, DH]]),
            )
            make_identity(nc, ident[:])
            # whole-tile memset: the v evac writes cols 0:64 of each head slot,
            # leaving col 64 = 1.0 (the denominator column), then *= pad
            nc.vector.memset(v_sb[:].bitcast(F32), 1.0)

            # ---------------- phase 1: projections ----------------
            with (
                tc.tile_pool(name="ph1", bufs=1) as ph1,
                tc.tile_pool(name="wpool", bufs=2) as wpool,
                tc.tile_pool(name="ps1", bufs=4, space="PSUM") as ps1,
            ):
                xT_sb = ph1.tile([128, KD, S], F32R)
                for kd in range(KD):
                    nc.gpsimd.dma_start(
                        out=xT_sb[:, kd, :],
                        in_=xT_d.ap().rearrange("(t p) s -> p t s", p=128)[:, kd, :],
                    )

                for w_d, b_sb, o_sb in ((wqT_d, bq_sb, qT_sb), (wkT_d, bk_sb, kT_sb)):
                    w_sb = wpool.tile([128, KD, DH], F32R, tag="w")
                    nc.gpsimd.dma_start(
                        out=w_sb, in_=w_d.ap().rearrange("(t p) n -> p t n", p=128)
                    )
                    for m in range(4):
                        for c in range(QC):
                            ps = ps1.tile([128, 512], F32, tag="ps1")
                            for kd in range(KD):
                                nc.tensor.matmul(
                                    ps[:],
                                    w_sb[:, kd, ts(m, 128)],
                                    xT_sb[:, kd, ts(c, 512)],
                                    start=(kd == 0),
                                    stop=(kd == KD - 1),
                                )
                            nc.vector.tensor_scalar_add(
                                o_sb[:, m, ts(c, 512)], ps[:], b_sb[:, m : m + 1]
                            )

                wv_sb = wpool.tile([128, KD, DH], F32R, tag="w")
                nc.gpsimd.dma_start(
                    out=wv_sb, in_=wvT_d.ap().rearrange("(t p) n -> p t n", p=128)
                )
                for t in range(KT):
                    ps = ps1.tile([128, 512], F32, tag="ps1")
                    for kd in range(KD):
                        nc.tensor.matmul(
                            ps[:],
                            xT_sb[:, kd, ts(t, 128)],
                            wv_sb[:, kd, :],
                            start=(kd == 0),
                            stop=(kd == KD - 1),
                        )
                    nc.vector.tensor_add(
                        v_sb[:, t, :, 0:HD],
                        ps.rearrange("p (h c) -> p h c", c=HD),
                        bv_bc.rearrange("p (h c) -> p h c", c=HD),
                    )
                    nc.vector.tensor_scalar_mul(
                        v_sb[:, t, :, :], v_sb[:, t, :, :], pad_sb[:, t : t + 1]
                    )

            # ---------------- phase 2: attention ----------------
            with (
                tc.tile_pool(name="epool", bufs=3) as epool,
                tc.tile_pool(name="ctxp", bufs=2) as ctxp,
                tc.tile_pool(name="octxp", bufs=3) as octxp,
                tc.tile_pool(name="rcp", bufs=3) as rcp,
                tc.tile_pool(name="psS", bufs=1, space="PSUM") as psS,
                tc.tile_pool(name="pvp", bufs=2, space="PSUM") as pvp,
                tc.tile_pool(name="trp", bufs=2, space="PSUM") as trp,
            ):
                for p in range(NPAIRS):
                    hA, hB = 2 * p, 2 * p + 1
                    for c in range(QC):
                        pvA = pvp.tile([128, 512], F32, tag="pv")
                        pvB = pvp.tile([128, 512], F32, tag="pv")
                        for g in range(KT // GS):
                            psA = psS.tile([128, GS * 512], F32, tag="psA")
                            psB = psS.tile([128, GS * 512], F32, tag="psB")
                            for j in range(GS):
                                kt = g * GS + j
                                nc.tensor.matmul(
                                    psA[:, ts(j, 512)],
                                    kT_sb[0:64, p, ts(kt, 128)],
                                    qT_sb[0:64, p, ts(c, 512)],
                                    start=True,
                                    stop=True,
                                )
                                nc.tensor.matmul(
                                    psB[:, ts(j, 512)],
                                    kT_sb[64:128, p, ts(kt, 128)],
                                    qT_sb[64:128, p, ts(c, 512)],
                                    start=True,
                                    stop=True,
                                )
                            eA = epool.tile([128, GS * 512], F32R, tag="eA")
                            eB = epool.tile([128, GS * 512], F32R, tag="eB")
                            nc.scalar.activation(
                                eA[:], psA[:], mybir.ActivationFunctionType.Exp,
                                scale=0.125,
                            )
                            nc.scalar.activation(
                                eB[:], psB[:], mybir.ActivationFunctionType.Exp,
                                scale=0.125,
                            )
                            for j in range(GS):
                                kt = g * GS + j
                                nc.tensor.matmul(
                                    pvA[0:65, :],
                                    v_sb[:, kt, hA, :],
                                    eA[:, ts(j, 512)],
                                    start=(kt == 0),
                                    stop=(kt == KT - 1),
                                )
                                nc.tensor.matmul(
                                    pvB[0:65, :],
                                    v_sb[:, kt, hB, :],
                                    eB[:, ts(j, 512)],
                                    start=(kt == 0),
                                    stop=(kt == KT - 1),
                                )
                        for head, pv in ((hA, pvA), (hB, pvB)):
                            ctxs = ctxp.tile([65, 512], F32, tag="ctxs")
                            nc.vector.tensor_copy(ctxs[:], pv[0:65, :])
                            tr = trp.tile([128, 512], F32, tag="tr")
                            octx = octxp.tile([128, 4, HD], F32, tag="octx")
                            rc = rcp.tile([128, 4], F32, tag="rc")
                            for blk in range(4):
                                nc.tensor.transpose(
                                    tr[:, ds(blk * 128, 65)],
                                    ctxs[:, ts(blk, 128)],
                                    ident[0:65, 0:65],
                                )
                                nc.vector.reciprocal(
                                    rc[:, blk : blk + 1],
                                    tr[:, ds(blk * 128 + HD, 1)],
                                )
                                nc.vector.tensor_scalar_mul(
                                    octx[:, blk, :],
                                    tr[:, ds(blk * 128, HD)],
                                    rc[:, blk : blk + 1],
                                )
                            nc.gpsimd.dma_start(
                                out=out_d[ds(c * 512, 512), ds(head * HD, HD)]
                                .rearrange("(blk p) d -> p blk d", p=128),
                                in_=octx[:],
                            )

    split_excess_waits(nc)
    return nc


_NC = None


def _get_nc():
    global _NC
    if _NC is None:
        _NC = build_nc()
    return _NC


def make_in_maps(hidden_states, pad, wq, bq, wk, bk, wv, bv):
    hidden_states = np.ascontiguousarray(np.asarray(hidden_states, dtype=np.float32))
    pad = np.asarray(pad, dtype=np.float32)
    in_maps = []
    for core in range(8):
        b, g = divmod(core, 2)
        sl = slice(512 * g, 512 * (g + 1))
        in_maps.append(
            {
                "xT": np.ascontiguousarray(hidden_states[b].T),
                "wqT": np.ascontiguousarray(np.asarray(wq, np.float32)[sl, :].T),
                "wkT": np.ascontiguousarray(np.asarray(wk, np.float32)[sl, :].T),
                "wvT": np.ascontiguousarray(np.asarray(wv, np.float32)[sl, :].T),
                "bq": np.ascontiguousarray(np.asarray(bq, np.float32)[sl]),
                "bk": np.ascontiguousarray(np.asarray(bk, np.float32)[sl]),
                "bv": np.ascontiguousarray(np.asarray(bv, np.float32)[sl]),
                "pad": np.ascontiguousarray(pad[b]),
            }
        )
    return in_maps


def assemble(results):
    out = np.empty((B, S, D), dtype=np.float32)
    for core in range(8):
        b, g = divmod(core, 2)
        out[b, :, 512 * g : 512 * (g + 1)] = results[core]["ctx"]
    return out


def kernel(hidden_states, pad, wq, bq, wk, bk, wv, bv):
    from concourse.bass_utils import run_bass_kernel_spmd

    nc = _get_nc()
    in_maps = make_in_maps(hidden_states, pad, wq, bq, wk, bk, wv, bv)
    res = run_bass_kernel_spmd(nc, in_maps, list(range(8)))
    return assemble(res.results)


# revision 17
# speedup vs baseline: 1.4783x; 1.4783x over previous
"""BertSelfAttention forward on 8 Trainium2 NeuronCores (Bass/Tile).

Problem: B=4, S=2048, D=1024, H=16 heads, head_dim=64.
Sharding: 8 cores = (batch b in 0..4) x (head-group g in 0..2); each core
computes heads [8g, 8g+8) of batch b => output slice out[b, :, 512g:512(g+1)].

Per-core kernel (all matmuls in float32r: full PE rate, ~2e-4 rel err):
  phase 1 (emission interleaved so attention can start as data lands):
    qT = (wq @ x.T) + bq      [dh=512, S]  (dh on partitions)
    kT = (wk @ x.T) + bk      [dh=512, S]
    v  = (x @ wv.T + bv)*pad  [S, dh=512]  stored per s-tile as
         [128, 8 heads, 65] with col 64 = pad column -> the PV matmul
         emits softmax denominators for free (M=65).
  phase 2: per head-pair (2p, 2p+1): the two heads' score matmuls are
    row-tiled on the PE (K=64 each at array rows 0:64 / 64:128) and run
    CONCURRENTLY (~116ns/MM measured); both write one [128, 1024] PSUM
    group (2 banks, double-buffered) so a single ScalarE activation does
    exp(0.125*scores) for both heads -> e[ks, 0:512]=head A, [512:1024]=B.
    ctxT_aug[65, qs] += v_aug.T @ e  accumulated over 16 k-tiles; then
    PE-transpose 128-blocks, divide by the denominator column
    (VectorE reciprocal + tensor_scalar), DMA out.

The pad mask is exact: exp(s + (1-pad)*-1e9) = exp(s)*pad, folded into the
v rows and the denominator column (both scaled by pad in phase 1).
"""

import numpy as np

import concourse.bass as bass
import concourse.tile as tile
from concourse import mybir
from concourse.bass import ds, ts
from concourse.masks import make_identity

B, S, D, H = 4, 2048, 1024, 16
HD = D // H          # 64
DH = 512             # head dims per core (8 heads)
NHEADS = 8           # heads per core
NPAIRS = 4           # head pairs per core
KT = 16              # k-tiles of 128 over S
QC = 4               # q-chunks of 512 over S
KD = 8               # contraction tiles of 128 over D

F32 = mybir.dt.float32
F32R = mybir.dt.float32r

MAX_WAITS = 1


def split_excess_waits(nc):
    """This walrus build accepts only one sync-wait per instruction; hoist
    excess waits onto preceding NoOps on the same engine."""
    for f in nc.m.functions:
        for bb in f.blocks:
            insts = list(bb.instructions)
            out = []
            changed = False
            for inst in insts:
                si = inst.sync_info
                if si is not None and si.on_wait and len(si.on_wait) > MAX_WAITS:
                    waits = list(si.on_wait)
                    changed = True
                    k = 0
                    while len(waits) - k > MAX_WAITS:
                        nop = mybir.InstNoOp(
                            name=f"{inst.name}-ws{k}", engine=inst.engine
                        )
                        nop.sync_info = mybir.SyncInfo(
                            on_wait=waits[k : k + MAX_WAITS], on_update=[]
                        )
                        out.append(nop)
                        k += MAX_WAITS
                    si.on_wait = waits[k:]
                out.append(inst)
            if changed:
                bb.instructions = out


def build_nc():
    nc = bass.Bass("TRN2", target_bir_lowering=False, debug=False, num_devices=8)

    xT_d = nc.declare_dram_parameter("xT", [D, S], F32R, isOutput=False)
    wqT_d = nc.declare_dram_parameter("wqT", [D, DH], F32R, isOutput=False)
    wkT_d = nc.declare_dram_parameter("wkT", [D, DH], F32R, isOutput=False)
    wvT_d = nc.declare_dram_parameter("wvT", [D, DH], F32R, isOutput=False)
    bq_d = nc.declare_dram_parameter("bq", [DH], F32, isOutput=False)
    bk_d = nc.declare_dram_parameter("bk", [DH], F32, isOutput=False)
    bv_d = nc.declare_dram_parameter("bv", [DH], F32, isOutput=False)
    pad_d = nc.declare_dram_parameter("pad", [S], F32, isOutput=False)
    out_d = nc.declare_dram_parameter("ctx", [S, DH], F32, isOutput=True)

    with tile.TileContext(nc) as tc:
        with (
            tc.tile_pool(name="persist", bufs=1) as persist,
            tc.tile_pool(name="small", bufs=1) as small,
            tc.tile_pool(name="phX", bufs=1) as phX,
        ):
            qT_sb = persist.tile([128, 4, S], F32R)
            kT_sb = persist.tile([128, 4, S], F32R)
            v_sb = persist.tile([128, KT, NHEADS, HD + 1], F32R)
            bq_sb = small.tile([128, 4], F32)
            bk_sb = small.tile([128, 4], F32)
            pad_sb = small.tile([128, KT], F32)
            bv_bc = small.tile([128, DH], F32)
            ident = small.tile([128, 128], F32)

            nc.gpsimd.dma_start(out=bq_sb, in_=bq_d.ap().rearrange("(m p) -> p m", p=128))
            nc.gpsimd.dma_start(out=bk_sb, in_=bk_d.ap().rearrange("(m p) -> p m", p=128))
            nc.gpsimd.dma_start(out=pad_sb, in_=pad_d.ap().rearrange("(t p) -> p t", p=128))
            nc.gpsimd.dma_start(
                out=bv_bc,
                in_=bass.AP(tensor=bv_d, offset=0, ap=[[0, 128], [1, DH]]),
            )
            make_identity(nc, ident[:])
            # whole-tile memset: the v evac writes cols 0:64 of each head slot,
            # leaving col 64 = 1.0 (denominator column), then *= pad
            nc.vector.memset(v_sb[:].bitcast(F32), 1.0)

            xT_r = xT_d.ap().rearrange("(t p) s -> p t s", p=128)
            wq_r = wqT_d.ap().rearrange("(t p) n -> p t n", p=128)
            wk_r = wkT_d.ap().rearrange("(t p) n -> p t n", p=128)
            wv_r = wvT_d.ap().rearrange("(t p) n -> p t n", p=128)

            xT_sb = phX.tile([128, KD, S], F32R)

            # ---------------- phase 1a: v projection ----------------
            with (
                tc.tile_pool(name="phV", bufs=1) as phV,
                tc.tile_pool(name="ps1", bufs=4, space="PSUM") as ps1,
            ):
                wv_sb = phV.tile([128, KD, DH], F32R)
                # kd-split DMAs so accumulation starts while data streams in;
                # xT on the sync engine (HWDGE) in parallel with gpsimd
                for kd in range(KD):
                    nc.sync.dma_start(out=xT_sb[:, kd, :], in_=xT_r[:, kd, :])
                    nc.gpsimd.dma_start(out=wv_sb[:, kd, :], in_=wv_r[:, kd, :])

                for t in range(KT):
                    ps = ps1.tile([128, 512], F32, tag="ps1")
                    for kd in range(KD):
                        nc.tensor.matmul(
                            ps[:],
                            xT_sb[:, kd, ts(t, 128)],
                            wv_sb[:, kd, :],
                            start=(kd == 0),
                            stop=(kd == KD - 1),
                        )
                    nc.vector.tensor_add(
                        v_sb[:, t, :, 0:HD],
                        ps.rearrange("p (h c) -> p h c", c=HD),
                        bv_bc.rearrange("p (h c) -> p h c", c=HD),
                    )
                    nc.vector.tensor_scalar_mul(
                        v_sb[:, t, :, :], v_sb[:, t, :, :], pad_sb[:, t : t + 1]
                    )

            # ------- phase 1b + 2: per-pair q/k projection + attention -------
            # (interleaved: later pairs' projections fill PE bubbles while the
            # ScalarE-paced attention of earlier pairs runs)
            with (
                tc.tile_pool(name="wpool", bufs=2) as wpool,
                tc.tile_pool(name="epool", bufs=3) as epool,
                tc.tile_pool(name="ctxp", bufs=2) as ctxp,
                tc.tile_pool(name="octxp", bufs=3) as octxp,
                tc.tile_pool(name="rcp", bufs=3) as rcp,
                tc.tile_pool(name="psS", bufs=2, space="PSUM") as psS,
                tc.tile_pool(name="pvp", bufs=2, space="PSUM") as pvp,
                tc.tile_pool(name="trp", bufs=1, space="PSUM") as trp,
                tc.tile_pool(name="psQ", bufs=1, space="PSUM") as psQ,
            ):

                def qk_proj(m, w_r, tag, b_sb, o_sb):
                    w_sb = wpool.tile([128, KD, 128], F32R, tag=tag)
                    for kd in range(KD):
                        nc.gpsimd.dma_start(
                            out=w_sb[:, kd, :], in_=w_r[:, kd, ts(m, 128)]
                        )
                    for c in range(QC):
                        ps = psQ.tile([128, 512], F32, tag="psq")
                        for kd in range(KD):
                            nc.tensor.matmul(
                                ps[:],
                                w_sb[:, kd, :],
                                xT_sb[:, kd, ts(c, 512)],
                                start=(kd == 0),
                                stop=(kd == KD - 1),
                            )
                        nc.vector.tensor_scalar_add(
                            o_sb[:, m, ts(c, 512)], ps[:], b_sb[:, m : m + 1]
                        )

                for p in range(NPAIRS):
                    qk_proj(p, wq_r, "wq", bq_sb, qT_sb)
                    qk_proj(p, wk_r, "wk", bk_sb, kT_sb)
                    hA, hB = 2 * p, 2 * p + 1
                    for c in range(QC):
                        pvA = pvp.tile([128, 512], F32, tag="pv")
                        pvB = pvp.tile([128, 512], F32, tag="pv")
                        for kt in range(KT):
                            grp = psS.tile([128, 1024], F32, tag="grp")
                            # row-tiled concurrent pair: head A on array rows
                            # 0:64, head B on rows 64:128
                            nc.tensor.matmul(
                                grp[:, 0:512],
                                kT_sb[0:64, p, ts(kt, 128)],
                                qT_sb[0:64, p, ts(c, 512)],
                                start=True,
                                stop=True,
                            )
                            nc.tensor.matmul(
                                grp[:, 512:1024],
                                kT_sb[64:128, p, ts(kt, 128)],
                                qT_sb[64:128, p, ts(c, 512)],
                                start=True,
                                stop=True,
                            )
                            e = epool.tile([128, 1024], F32R, tag="e")
                            nc.scalar.activation(
                                e[:], grp[:], mybir.ActivationFunctionType.Exp,
                                scale=0.125,
                            )
                            nc.tensor.matmul(
                                pvA[0:65, :],
                                v_sb[:, kt, hA, :],
                                e[:, 0:512],
                                start=(kt == 0),
                                stop=(kt == KT - 1),
                            )
                            nc.tensor.matmul(
                                pvB[0:65, :],
                                v_sb[:, kt, hB, :],
                                e[:, 512:1024],
                                start=(kt == 0),
                                stop=(kt == KT - 1),
                            )
                        for head, pv in ((hA, pvA), (hB, pvB)):
                            ctxs = ctxp.tile([65, 512], F32, tag="ctxs")
                            nc.vector.tensor_copy(ctxs[:], pv[0:65, :])
                            tr = trp.tile([128, 512], F32, tag="tr")
                            octx = octxp.tile([128, 4, HD], F32, tag="octx")
                            rc = rcp.tile([128, 4], F32, tag="rc")
                            for blk in range(4):
                                nc.tensor.transpose(
                                    tr[:, ds(blk * 128, 65)],
                                    ctxs[:, ts(blk, 128)],
                                    ident[0:65, 0:65],
                                )
                                nc.vector.reciprocal(
                                    rc[:, blk : blk + 1],
                                    tr[:, ds(blk * 128 + HD, 1)],
                                )
                                nc.vector.tensor_scalar_mul(
                                    octx[:, blk, :],
                                    tr[:, ds(blk * 128, HD)],
                                    rc[:, blk : blk + 1],
                                )
                            nc.gpsimd.dma_start(
                                out=out_d[ds(c * 512, 512), ds(head * HD, HD)]
                                .rearrange("(blk p) d -> p blk d", p=128),
                                in_=octx[:],
                            )

    split_excess_waits(nc)
    return nc


_NC = None


def _get_nc():
    global _NC
    if _NC is None:
        _NC = build_nc()
    return _NC


def make_in_maps(hidden_states, pad, wq, bq, wk, bk, wv, bv):
    hidden_states = np.ascontiguousarray(np.asarray(hidden_states, dtype=np.float32))
    pad = np.asarray(pad, dtype=np.float32)
    in_maps = []
    for core in range(8):
        b, g = divmod(core, 2)
        sl = slice(512 * g, 512 * (g + 1))
        in_maps.append(
            {
                "xT": np.ascontiguousarray(hidden_states[b].T),
                "wqT": np.ascontiguousarray(np.asarray(wq, np.float32)[sl, :].T),
                "wkT": np.ascontiguousarray(np.asarray(wk, np.float32)[sl, :].T),
                "wvT": np.ascontiguousarray(np.asarray(wv, np.float32)[sl, :].T),
                "bq": np.ascontiguousarray(np.asarray(bq, np.float32)[sl]),
                "bk": np.ascontiguousarray(np.asarray(bk, np.float32)[sl]),
                "bv": np.ascontiguousarray(np.asarray(bv, np.float32)[sl]),
                "pad": np.ascontiguousarray(pad[b]),
            }
        )
    return in_maps


def assemble(results):
    out = np.empty((B, S, D), dtype=np.float32)
    for core in range(8):
        b, g = divmod(core, 2)
        out[b, :, 512 * g : 512 * (g + 1)] = results[core]["ctx"]
    return out


def kernel(hidden_states, pad, wq, bq, wk, bk, wv, bv):
    from concourse.bass_utils import run_bass_kernel_spmd

    nc = _get_nc()
    in_maps = make_in_maps(hidden_states, pad, wq, bq, wk, bk, wv, bv)
    res = run_bass_kernel_spmd(nc, in_maps, list(range(8)))
    return assemble(res.results)
